# revision 30
# baseline (speedup 1.0000x reference)
"""JKNet (5-layer GCN + JumpingKnowledge-max + linear head) on 8 Trainium2 cores.

Strategy (dst-sharded message passing, v4):
  - Nodes sharded across 8 cores (12500 per core) and PACKED into 98 tiles of
    <=128 nodes each (greedy bin-packing with per-tile node slack) so that each
    tile's incoming-edge count per source bucket fits a shared (tile, bucket)
    chunk template with minimal 128-alignment padding.  Node-indexed tensors
    live in a permuted "slot id" (sid) space of 12544 slots per core (100352
    total); the host unpermutes the final output.
  - Self-loop terms are handled densely (not as edges): a persistent SBUF
    buffer hs = h * selfnorm (feature-major) feeds a second accumulating
    W-matmul per tile.
  - Gathers: gpsimd.dma_gather (int16 idxs, 4 source buckets of 25088 rows),
    in <=1024-row calls (SWDGE descriptor-ring cap), round-robin over 4 SWDGE
    queues.  Descriptor generation (~2.6ns/row on the Q7s) is the kernel's
    critical path; everything else is overlapped under it.
  - One-hot aggregation matrices S[e, dst] = norm_e * onehot(dst_e) are
    PRECOMPUTED ON THE HOST, stored in HBM, streamed per tile-group per layer.
  - bf16 everywhere (PE 1 cyc/row); PSUM accumulates fp32.
  - Per tile: q^T = sum_chunks msgs^T @ S (PE); ph = W^T q + W^T hs (one PSUM
    chain); fused BN+ReLU on ACT; JK running max (DVE); hs update (DVE);
    PE transpose; DMA to the AllGather input.  bf16 AllGather per layer.
  - Last layer: BN on DVE with ReLU absorbed into the JK max (hmax >= 0), and
    the head's logits+Exp phase fused into the tile loop (Act does only Exp,
    one activation-table load).  Softmax finishes in two short tail phases.
"""

import math
import os

import numpy as np
from ml_dtypes import bfloat16

import concourse.bass as bass
import concourse.mybir as mybir
import concourse.tile as tile
from concourse import bacc
from concourse.bass_utils import run_bass_kernel_spmd
from concourse.masks import make_identity

P = 128            # partitions / feature dim / edge-chunk size
NCORES = 8
BN_EPS = 1e-5
NB = 4             # source buckets (int16 gather indices: bucket < 32768 rows)
GT = 6             # dst tiles per gather group
CALL_CHUNKS = 8    # chunks per dma_gather call (1024 rows = SWDGE ring cap)
SLACK_CHUNKS = 2   # per-bucket template slack beyond the max core load


# ---------------------------------------------------------------- host prep
def _pack_core(cnt, cap_node, C):
    """Greedy bin-packing of nodes into tiles.

    cnt: [n_nodes_core, NB] per-node per-bucket in-edge counts
    cap_node: node capacity per tile (scalar)
    C: [n_tiles, NB] edge capacity per tile/bucket (may be exceeded; caller
       re-derives the realized template)
    Returns assign: [n_nodes_core] tile id per node.
    """
    n, t_tiles = cnt.shape[0], C.shape[0]
    order = np.argsort(-cnt.sum(1), kind="stable")
    ncnt = np.zeros(t_tiles, dtype=np.int64)
    load = np.zeros((t_tiles, NB), dtype=np.int64)
    assign = np.zeros(n, dtype=np.int64)
    Cf = C.astype(np.float64)
    for i in order:
        ci = cnt[i]
        open_ = ncnt < cap_node
        fits = open_ & np.all(load + ci <= C, axis=1)
        if fits.any():
            cand = np.flatnonzero(fits)
            # balanced fill: place into the least-loaded feasible bin
            ratio = ((load[cand] + ci) / Cf[cand]).max(axis=1)
            pick = cand[np.argmin(ratio)]
        else:
            cand = np.flatnonzero(open_)
            over = np.maximum(load[cand] + ci - C[cand], 0).sum(axis=1)
            pick = cand[np.argmin(over)]
        assign[i] = pick
        ncnt[pick] += 1
        load[pick] += ci

    # repair: move nodes out of overflowing (tile, bucket) segments
    for _ in range(4000):
        over = load - C
        tb = np.unravel_index(np.argmax(over), over.shape)
        if over[tb] <= 0:
            break
        t_bad, b_bad = tb
        members = np.flatnonzero(assign == t_bad)
        cand_nodes = members[cnt[members, b_bad] > 0]
        if len(cand_nodes) == 0:
            break
        big = cand_nodes[np.argsort(cnt[cand_nodes, b_bad])]
        moved = False
        for node in big[::-1][:20]:
            ci = cnt[node]
            dest_ok = (ncnt < cap_node) & np.all(load + ci <= C, axis=1)
            dest_ok[t_bad] = False
            if dest_ok.any():
                d = np.flatnonzero(dest_ok)
                pick = d[np.argmax(((load[d] + ci) / Cf[d]).max(axis=1))]
                assign[node] = pick
                load[t_bad] -= ci
                load[pick] += ci
                ncnt[t_bad] -= 1
                ncnt[pick] += 1
                moved = True
                break
        if not moved:
            break
    return assign


def preprocess_edges(x, edge_index, n_nodes, ncores=NCORES):
    row = np.asarray(edge_index[0], dtype=np.int64)   # dst
    col = np.asarray(edge_index[1], dtype=np.int64)   # src
    deg = np.bincount(row, minlength=n_nodes).astype(np.float64) + 1.0
    dinv = 1.0 / np.sqrt(deg)
    enrm = (dinv[row] * dinv[col]).astype(np.float32)
    selfn = (dinv * dinv).astype(np.float32)

    sh = n_nodes // ncores                 # real nodes per core (12500)
    t_tiles = math.ceil(sh / P)            # 98
    sn_core = t_tiles * P                  # slot-sids per core (12544)
    n_slots = sn_core * ncores             # table rows (100352)
    bucket = sn_core * 2                   # 25088 (< 2^15)

    dcore = row // sh
    b_of = (col // sh) // 2                # src bucket == src core pair

    # shared chunk template sized for the heaviest (core, bucket) edge load
    ecnt_cb = np.zeros((ncores, NB), dtype=np.int64)
    np.add.at(ecnt_cb, (dcore, b_of), 1)
    need = math.ceil(ecnt_cb.max() / P) + SLACK_CHUNKS
    base = np.full((t_tiles, NB), 2, dtype=np.int64)
    bonus_per_b = max(0, need - 2 * t_tiles)
    for b_ in range(NB):
        cand = list(range(t_tiles))[b_::NB] \
            + [t for t in range(t_tiles) if t % NB != b_]
        for t in cand[:bonus_per_b]:
            base[t, b_] += 1

    # pack each core -> sid permutation (sid = core*12544 + tile*128 + slot)
    perm = np.full(n_slots, -1, dtype=np.int64)    # sid -> orig node (-1 empty)
    sid_of = np.zeros(n_nodes, dtype=np.int64)     # orig node -> sid
    loads = np.zeros((ncores, t_tiles, NB), dtype=np.int64)
    edge_core = [None] * ncores
    for c in range(ncores):
        m = dcore == c
        nodes0 = c * sh
        cnt = np.zeros((sh, NB), dtype=np.int64)
        np.add.at(cnt, (row[m] - nodes0, b_of[m]), 1)
        assign = _pack_core(cnt, P, base * P)
        order = np.argsort(assign, kind="stable")
        tcnt = np.bincount(assign, minlength=t_tiles)
        assert (tcnt <= P).all()
        ofs = np.concatenate([[0], np.cumsum(tcnt)])[:-1]
        within = np.arange(sh) - ofs[assign[order]]
        sid_local = np.zeros(sh, dtype=np.int64)
        sid_local[order] = assign[order] * P + within
        sid_of[nodes0 + np.arange(sh)] = c * sn_core + sid_local
        perm[c * sn_core + sid_local] = nodes0 + np.arange(sh)
        lt = np.zeros((t_tiles, NB), dtype=np.int64)
        np.add.at(lt, (assign[row[m] - nodes0], b_of[m]), 1)
        loads[c] = lt
        edge_core[c] = m

    seg_chunks = np.maximum(
        np.ceil(loads.max(axis=0) / P).astype(np.int64), 1)  # [t_tiles, NB]

    # groups of GT tiles, but the last two groups are small (2 tiles) so the
    # end-of-layer compute tail behind the final gathers is short
    bounds = list(range(0, t_tiles, GT))
    groups = [list(range(b, min(b + GT, t_tiles))) for b in bounds]
    if len(groups[-1]) > 2:
        tail = groups.pop()
        groups.extend([tail[:-2], tail[-2:]] if len(tail) > 2 else [tail])
    else:
        pen = groups[-2]
        if len(pen) > 2:
            groups[-2] = pen[:-2]
            groups.insert(len(groups) - 1, pen[-2:])
    groups = [g for g in groups if g]
    call_info = []        # per group: list of (bucket, chunk_off, n_chunks)
    tile_chunks = [[] for _ in range(t_tiles)]
    gmax = 0
    off = 0
    for grp in groups:
        ring_col = 0
        calls = []
        for b in range(NB):
            nb_chunks = int(seg_chunks[list(grp), b].sum())
            if nb_chunks == 0:
                continue
            calls.append((b, off, nb_chunks))
            for t in grp:
                for _ in range(int(seg_chunks[t, b])):
                    tile_chunks[t].append((off, ring_col))
                    off += 1
                    ring_col += 1
        call_info.append(calls)
        gmax = max(gmax, ring_col)
    totc = off
    tot = totc * P

    chunk_off = np.zeros((t_tiles, NB), dtype=np.int64)
    pos = 0
    for grp in groups:
        for b in range(NB):
            for t in grp:
                chunk_off[t, b] = pos
                pos += int(seg_chunks[t, b])

    xf = np.asarray(x, dtype=np.float32)
    x_sid = np.zeros((n_slots, P), dtype=bfloat16)
    valid = perm >= 0
    x_sid[valid] = xf[perm[valid]].astype(bfloat16)

    per_core = []
    for c in range(ncores):
        m = edge_core[c]
        er, ec, ew, eb = row[m], col[m], enrm[m], b_of[m]
        d_sid = sid_of[er] - c * sn_core
        dt_ = d_sid // P
        dslot = d_sid % P
        sloc = sid_of[ec] - eb * bucket
        assert (sloc >= 0).all() and (sloc < bucket).all()
        order = np.lexsort((sloc, eb, dt_))
        dt_, dslot, eb, sloc, ew = (dt_[order], dslot[order], eb[order],
                                    sloc[order], ew[order])
        cnt2 = np.zeros((t_tiles, NB), dtype=np.int64)
        np.add.at(cnt2, (dt_, eb), 1)
        starts = np.concatenate([[0], np.cumsum(cnt2.flatten())])

        idx_f = np.zeros(tot, dtype=np.int16)
        r_valid, r_slot, r_nrm = [], [], []
        for t in range(t_tiles):
            for b in range(NB):
                kt = int(seg_chunks[t, b])
                if kt == 0:
                    continue
                a0 = int(starts[t * NB + b])
                n = int(cnt2[t, b])
                p0 = int(chunk_off[t, b]) * P
                assert n <= kt * P, (t, b, n, kt)
                if n:
                    idx_f[p0:p0 + n] = sloc[a0:a0 + n]
                    idx_f[p0 + n:p0 + kt * P] = sloc[a0 + n - 1]
                    r_valid.append(np.arange(p0, p0 + n))
                    r_slot.append(dslot[a0:a0 + n])
                    r_nrm.append(ew[a0:a0 + n])
        rv = np.concatenate(r_valid)
        rs = np.concatenate(r_slot).astype(np.int64)
        rw = np.concatenate(r_nrm)
        s_dram = np.zeros((P, totc * P), dtype=bfloat16)
        s_dram[rv % P, (rv // P) * P + rs] = rw.astype(bfloat16)

        idx_w = idx_f.reshape(tot // 16, 16).T
        idx_w = np.tile(idx_w, (8, 1)).astype(np.int16)

        # hs0 = x_own^T * selfnorm ; snb = selfnorm broadcast (feature-major)
        myperm = perm[c * sn_core:(c + 1) * sn_core]
        vmask = myperm >= 0
        vcols = np.flatnonzero(vmask)
        vorig = myperm[vcols]
        hs0 = np.zeros((P, sn_core), dtype=np.float32)
        snb = np.zeros((P, sn_core), dtype=np.float32)
        hs0[:, vcols] = (xf[vorig] * selfn[vorig][:, None]).T
        snb[:, vcols] = np.broadcast_to(selfn[vorig], (P, len(vcols)))
        per_core.append({
            "eidx": np.ascontiguousarray(idx_w),
            "s_dram": np.ascontiguousarray(s_dram),
            "hs0": np.ascontiguousarray(hs0.astype(bfloat16)),
            "snb": np.ascontiguousarray(snb.astype(bfloat16)),
        })

    plan = {
        "t_tiles": t_tiles, "bucket": bucket, "groups": groups,
        "call_info": call_info, "tile_chunks": tile_chunks,
        "totc": totc, "gmax": gmax, "sh": sh, "sn_core": sn_core,
        "n_slots": n_slots, "perm": perm,
    }
    return x_sid, per_core, plan


# ---------------------------------------------------------------- program
def build_program(n_layers, n_cls, plan, ncores=NCORES):
    f32 = mybir.dt.float32
    bf16 = mybir.dt.bfloat16
    i16 = mybir.dt.int16
    t_tiles = plan["t_tiles"]
    bucket = plan["bucket"]
    groups = plan["groups"]
    call_info = plan["call_info"]
    tile_chunks = plan["tile_chunks"]
    totc = plan["totc"]
    gmax = plan["gmax"]
    sn_core = plan["sn_core"]
    n_slots = plan["n_slots"]
    tot = totc * P

    nc = bacc.Bacc("TRN2", target_bir_lowering=False, debug=False,
                   num_devices=ncores, num_swdge_queues=4)
    x_t = nc.dram_tensor("x", [n_slots, P], bf16, kind="ExternalInput")
    idx_t = nc.dram_tensor("eidx", [P, tot // 16], i16, kind="ExternalInput")
    s_t = nc.dram_tensor("s_dram", [P, totc * P], bf16, kind="ExternalInput")
    hs0_t = nc.dram_tensor("hs0", [P, sn_core], bf16, kind="ExternalInput")
    snb_t = nc.dram_tensor("snb", [P, sn_core], bf16, kind="ExternalInput")
    w_t = nc.dram_tensor("conv_w", [n_layers, P, P], bf16, kind="ExternalInput")
    bns_t = nc.dram_tensor("bn_scale", [n_layers, P], f32, kind="ExternalInput")
    bnh_t = nc.dram_tensor("bn_shift", [n_layers, P], f32, kind="ExternalInput")
    lw_t = nc.dram_tensor("lin_w", [P, n_cls], bf16, kind="ExternalInput")
    lb_t = nc.dram_tensor("lin_b_rep", [P, n_cls], f32, kind="ExternalInput")
    out_t = nc.dram_tensor("out", [sn_core, n_cls], f32, kind="ExternalOutput")

    ag_in = [nc.dram_tensor(f"ag_in{l}", [sn_core, P], bf16)
             for l in range(n_layers - 1)]
    hbuf = [nc.dram_tensor(f"hbuf{l}", [n_slots, P], bf16, addr_space="Shared")
            for l in range(n_layers - 1)]
    # tiny barrier collective: re-aligns cores mid-layer so the real
    # AllGather's rendezvous wait is short (content is irrelevant)
    sync_in = nc.dram_tensor("sync_in", [P // ncores, P], bf16)
    sync_out = nc.dram_tensor("sync_out", [P, P], bf16, addr_space="Shared")
    rgroups = [list(range(ncores))]
    AF = mybir.ActivationFunctionType
    OP = mybir.AluOpType

    with tile.TileContext(nc) as tc:
        with tc.tile_pool(name="const", bufs=1) as cpool, \
             tc.tile_pool(name="edges", bufs=1) as epool, \
             tc.tile_pool(name="msgs", bufs=3) as mpool, \
             tc.tile_pool(name="sring", bufs=2) as spool, \
             tc.tile_pool(name="work", bufs=3) as wpool, \
             tc.tile_pool(name="psum", bufs=2, space="PSUM") as pspool:

            # -------- resident data + constants
            idx_sb = epool.tile([P, tot // 16], i16)
            # split the idx upload so the first groups' gathers start sooner
            c_head = sum(nbc for (_, _, nbc) in call_info[0]) * 8
            nc.sync.dma_start(out=idx_sb[:, :c_head], in_=idx_t[:, :c_head])
            nc.sync.dma_start(out=idx_sb[:, c_head:], in_=idx_t[:, c_head:])
            hsbuf = epool.tile([P, sn_core], bf16)
            nc.sync.dma_start(out=hsbuf[:], in_=hs0_t[:])
            snb_sb = epool.tile([P, sn_core], bf16)
            nc.sync.dma_start(out=snb_sb[:], in_=snb_t[:])

            ident = cpool.tile([P, P], bf16)
            make_identity(nc, ident[:])

            w_sb = []
            for l in range(n_layers):
                wl = cpool.tile([P, P], bf16, tag=f"w{l}")
                nc.sync.dma_start(out=wl[:], in_=w_t[l, :, :])
                w_sb.append(wl)
            lw_sb = cpool.tile([P, n_cls], bf16)
            nc.sync.dma_start(out=lw_sb[:], in_=lw_t[:])
            lb_sb = cpool.tile([P, n_cls], f32)
            nc.sync.dma_start(out=lb_sb[:], in_=lb_t[:])

            s_sb, sh_sb = [], []
            for l in range(n_layers):
                s_ = cpool.tile([P, 1], f32, tag=f"bns{l}")
                h_ = cpool.tile([P, 1], f32, tag=f"bnh{l}")
                nc.sync.dma_start(out=s_[:], in_=bns_t[l, :, None])
                nc.sync.dma_start(out=h_[:], in_=bnh_t[l, :, None])
                s_sb.append(s_)
                sh_sb.append(h_)

            hmax = epool.tile([P, sn_core], bf16)
            nc.vector.memset(hmax[:], 0.0)

            zbuf = epool.tile([P, t_tiles * n_cls], f32)
            ezbuf = epool.tile([P, t_tiles * n_cls], bf16)
            nmbuf = epool.tile([P, t_tiles], f32)
            lsbuf = epool.tile([P, t_tiles], f32)

            # -------- layers
            qctr = [0]
            last = n_layers - 1
            for l in range(n_layers):
                table = x_t if l == 0 else hbuf[l - 1]
                gc_base = 0
                g_sync = len(groups) - 4
                for gi, grp in enumerate(groups):
                    g_chunks = sum(nbc for (_, _, nbc) in call_info[gi])
                    ring = mpool.tile([P, gmax * P], bf16, tag="msgs")
                    if l < last and gi == g_sync:
                        nc.gpsimd.collective_compute(
                            "AllGather", OP.bypass, replica_groups=rgroups,
                            ins=[sync_in[:]], outs=[sync_out[:]])
                    if l < last and gi == g_sync + 1:
                        # gate this group's gathers on the barrier output
                        nc.sync.dma_start(out=ring[:, 0:1],
                                          in_=sync_out[:, 0:1])
                    s_ring = spool.tile([P, gmax * P], bf16, tag="S")
                    nc.sync.dma_start(
                        out=s_ring[:, :g_chunks * P],
                        in_=s_t[:, gc_base * P:(gc_base + g_chunks) * P])
                    rc = 0
                    for (b, c_off, nb_chunks) in call_info[gi]:
                        done = 0
                        while done < nb_chunks:
                            n8 = min(CALL_CHUNKS, nb_chunks - done)
                            ni = n8 * P
                            r0 = rc + done
                            c0 = c_off + done
                            nc.gpsimd.dma_gather(
                                out_ap=ring[:, r0 * P:(r0 + n8) * P]
                                    .rearrange("p (k f) -> p k f", k=n8),
                                in_ap=table[b * bucket:(b + 1) * bucket, :],
                                idxs_ap=idx_sb[:, c0 * 8:(c0 + n8) * 8],
                                num_idxs=ni, num_idxs_reg=ni, elem_size=P,
                                queue_num=qctr[0] % 4)
                            qctr[0] += 1
                            done += n8
                        rc += nb_chunks
                    for t in grp:
                        kt = len(tile_chunks[t])
                        psq = pspool.tile([P, P], f32, tag="q", space="PSUM")
                        for k, (gc, ring_col) in enumerate(tile_chunks[t]):
                            nc.tensor.matmul(
                                psq[:],
                                lhsT=ring[:, ring_col * P:(ring_col + 1) * P],
                                rhs=s_ring[:, (gc - gc_base) * P:
                                           (gc - gc_base + 1) * P],
                                start=(k == 0), stop=(k == kt - 1))
                        q_sb = wpool.tile([P, P], bf16, tag="qT")
                        if l < last:
                            nc.scalar.copy(q_sb[:], psq[:])
                        else:
                            nc.vector.tensor_copy(q_sb[:], psq[:])
                        ph = pspool.tile([P, P], f32, tag="h", space="PSUM")
                        nc.tensor.matmul(ph[:], lhsT=w_sb[l][:], rhs=q_sb[:],
                                         start=True, stop=False)
                        nc.tensor.matmul(ph[:], lhsT=w_sb[l][:],
                                         rhs=hsbuf[:, t * P:(t + 1) * P],
                                         start=False, stop=True)
                        ts = slice(t * P, (t + 1) * P)
                        if l < last:
                            h_t = wpool.tile([P, P], bf16, tag="hT")
                            nc.scalar.activation(h_t[:], ph[:], AF.Relu,
                                                 bias=sh_sb[l][:, :1],
                                                 scale=s_sb[l][:, :1])
                            nc.vector.tensor_tensor(
                                out=hmax[:, ts], in0=hmax[:, ts],
                                in1=h_t[:], op=OP.max)
                            nc.vector.tensor_tensor(
                                out=hsbuf[:, ts], in0=h_t[:],
                                in1=snb_sb[:, ts], op=OP.mult)
                            pt = pspool.tile([P, P], bf16, tag="t", space="PSUM")
                            nc.tensor.transpose(pt[:], h_t[:], ident[:])
                            hn = wpool.tile([P, P], bf16, tag="hn")
                            nc.vector.tensor_copy(hn[:], pt[:])
                            nc.sync.dma_start(
                                out=ag_in[l][t * P:(t + 1) * P, :], in_=hn[:])
                        else:
                            # BN on DVE (ReLU absorbed by max vs hmax>=0), then
                            # head phase 1 fused: logits, bias, max, Exp
                            h_t = wpool.tile([P, P], bf16, tag="hT")
                            nc.vector.tensor_scalar(
                                out=h_t[:], in0=ph[:],
                                scalar1=s_sb[l][:, :1], scalar2=sh_sb[l][:, :1],
                                op0=OP.mult, op1=OP.add)
                            nc.vector.tensor_tensor(
                                out=hmax[:, ts], in0=hmax[:, ts],
                                in1=h_t[:], op=OP.max)
                            po = pspool.tile([P, n_cls], f32, tag="po",
                                             space="PSUM")
                            nc.tensor.matmul(po[:], lhsT=hmax[:, ts],
                                             rhs=lw_sb[:], start=True, stop=True)
                            z = zbuf[:, t * n_cls:(t + 1) * n_cls]
                            nc.vector.tensor_tensor(out=z, in0=po[:],
                                                    in1=lb_sb[:], op=OP.add)
                            nc.vector.reduce_max(nmbuf[:, t:t + 1], z,
                                                 axis=mybir.AxisListType.X,
                                                 negate=True)
                            nc.scalar.activation(
                                ezbuf[:, t * n_cls:(t + 1) * n_cls], z,
                                AF.Exp, bias=nmbuf[:, t:t + 1], scale=1.0)
                    gc_base += g_chunks
                if l < last:
                    nc.gpsimd.collective_compute(
                        "AllGather", OP.bypass, replica_groups=rgroups,
                        ins=[ag_in[l][:]], outs=[hbuf[l][:]])

            # -------- head tail: batched sum / Ln / assemble / one DMA
            ssbuf = epool.tile([P, t_tiles], f32)
            nc.vector.reduce_sum(
                ssbuf[:], ezbuf[:].rearrange("p (t c) -> p t c", t=t_tiles),
                axis=mybir.AxisListType.X)
            nc.scalar.activation(lsbuf[:], ssbuf[:], AF.Ln)
            nlbuf = epool.tile([P, t_tiles], f32)
            nc.vector.tensor_tensor(out=nlbuf[:], in0=nmbuf[:], in1=lsbuf[:],
                                    op=OP.subtract)
            ozbuf = epool.tile([P, t_tiles * n_cls], f32)
            nc.vector.tensor_tensor(
                out=ozbuf[:].rearrange("p (t c) -> p t c", t=t_tiles),
                in0=zbuf[:].rearrange("p (t c) -> p t c", t=t_tiles),
                in1=nlbuf[:].unsqueeze(2).to_broadcast((P, t_tiles, n_cls)),
                op=OP.add)
            nc.sync.dma_start(
                out=out_t[:].rearrange("(t p) c -> p t c", p=P),
                in_=ozbuf[:].rearrange("p (t c) -> p t c", t=t_tiles))

    nc.compile()
    return nc


# ---------------------------------------------------------------- runner
def run(x, edge_index, conv_w, conv_b, bn_gamma, bn_beta, bn_mean, bn_var,
        lin_w, lin_b, *, trace=False):
    n_nodes, d = x.shape
    n_layers = conv_w.shape[0]
    n_cls = lin_w.shape[1]
    assert d == P and n_nodes % NCORES == 0

    x_sid, per_core, plan = preprocess_edges(x, edge_index, n_nodes)
    nc = build_program(n_layers, n_cls, plan)

    g = np.asarray(bn_gamma, dtype=np.float64)
    be = np.asarray(bn_beta, dtype=np.float64)
    mu = np.asarray(bn_mean, dtype=np.float64)
    va = np.asarray(bn_var, dtype=np.float64)
    cb = np.asarray(conv_b, dtype=np.float64)
    scale = (g / np.sqrt(va + BN_EPS)).astype(np.float32)
    shift = (scale * (cb - mu) + be).astype(np.float32)

    shared = {
        "x": x_sid,
        "conv_w": np.ascontiguousarray(
            np.asarray(conv_w, dtype=np.float32).astype(bfloat16)),
        "bn_scale": np.ascontiguousarray(scale),
        "bn_shift": np.ascontiguousarray(shift),
        "lin_w": np.ascontiguousarray(
            np.asarray(lin_w, dtype=np.float32).astype(bfloat16)),
        "lin_b_rep": np.ascontiguousarray(
            np.broadcast_to(np.asarray(lin_b, dtype=np.float32),
                            (P, n_cls))).astype(np.float32),
    }
    in_maps = [dict(shared, **per_core[c]) for c in range(NCORES)]
    res = run_bass_kernel_spmd(nc, in_maps, list(range(NCORES)), trace=trace)
    out_sid = np.concatenate([np.asarray(res.results[c]["out"])
                              for c in range(NCORES)], axis=0)
    perm = plan["perm"]
    valid = perm >= 0
    out = np.empty((n_nodes, n_cls), dtype=out_sid.dtype)
    out[perm[valid]] = out_sid[valid]
    return out, res


def kernel(x, edge_index, conv_w, conv_b, bn_gamma, bn_beta, bn_mean, bn_var,
           lin_w, lin_b):
    out, _ = run(x, edge_index, conv_w, conv_b, bn_gamma, bn_beta,
                 bn_mean, bn_var, lin_w, lin_b,
                 trace=bool(int(os.environ.get("JKNET_TRACE", "0"))))
    return out
